# revision 13
# baseline (speedup 1.0000x reference)
"""Batch-parallel attention (B=8, Lq=Lkv=2048, D=DV=128) for 8 Trainium2 NeuronCores.

Sharding: batch dim across the 8 cores (data parallel, no cross-core comms).
Each core computes, for its (2048, 128) Q/K/V slice:
    S = Q @ K^T * (1/sqrt(D));  W = softmax(S);  ctx = W @ V
and writes both W (2048, 2048) and ctx (2048, 128) in natural layout.

Per-core plan (matmuls in float32r / tf32 operand precision with fp32 PSUM
accumulation — full PE rate at N=512, ~4e-4 rel err vs the fp32 reference.
The two QK^T orientations are recomputed rather than transposed: PE streams
512 cols per 216 ns in a matmul vs 128 cols per ~275 ns in transpose mode,
so recompute is ~5x cheaper than transposing the 16 MiB score matrix):
  phase 0: load Q,K,V; build Q^T, K^T ([D, L] layout) via 32 PE transposes.
  phase 1 (S^T layout, Lq chunks of 512): S^T = (K^T)^T_blk @ Q^T -> exp on
    ScalarE (PSUM->SBUF) -> ctx^T += V_blk^T @ expS^T accumulated over the 16
    kv blocks in PSUM.  Produces unnormalized ctx^T [DV, L] in SBUF.
  phase 2 (natural layout, 16 Lq tiles): S = (Q^T)^T_blk @ K^T recomputed,
    exp with fused per-partition row-sum (accum_out) -> reciprocal ->
    tensor_scalar normalize -> DMA out 1 MiB weight tiles.
  phase 3: PE-transpose ctx^T tiles back to natural, normalize by the same
    reciprocals, DMA out.

softmax skips the max-subtraction: scores are ~N(0,1) after scaling, so
exp() stays in [e-6, e+6] — no overflow risk in fp32, and exp(x)/sum(exp(x))
is algebraically identical to the max-subtracted form.
"""

import math

import numpy as np

import concourse.mybir as mybir
import concourse.tile as tile
from concourse import bacc
from concourse import bass_utils
from concourse.bass import ds, ts
from concourse.masks import make_identity

B, L, D = 8, 2048, 128
N_CORES = 8
NT = L // 128          # 16 128-row tiles
SCALE = 1.0 / math.sqrt(D)
F32 = mybir.dt.float32
F32R = mybir.dt.float32r


def r(ap):
    """Reinterpret an fp32 AP as float32r (full-rate PE matmul mode)."""
    return ap.bitcast(F32R)


def build_attention_nc():
    nc = bacc.Bacc("TRN2", target_bir_lowering=False, debug=False,
                   num_devices=N_CORES)

    q_d = nc.dram_tensor("q", (L, D), F32, kind="ExternalInput")
    k_d = nc.dram_tensor("k", (L, D), F32, kind="ExternalInput")
    v_d = nc.dram_tensor("v", (L, D), F32, kind="ExternalInput")
    w_d = nc.dram_tensor("w", (L, L), F32, kind="ExternalOutput")
    ctx_d = nc.dram_tensor("ctx", (L, D), F32, kind="ExternalOutput")

    with tile.TileContext(nc) as tc:
        with (
            # persistent SBUF (one buf each)
            tc.tile_pool(name="persist", bufs=1) as persist,
            # cycling SBUF pools
            tc.tile_pool(name="expst", bufs=3) as expst_pool,
            tc.tile_pool(name="wout", bufs=2) as w_pool,
            tc.tile_pool(name="csml", bufs=2) as cs_pool,
            tc.tile_pool(name="dens", bufs=4) as den_pool,
            # PSUM pools: 2 + 2 + 4 = 8 banks total
            tc.tile_pool(name="ptp", bufs=2, space="PSUM") as ptp,      # 2 banks
            tc.tile_pool(name="pctxT", bufs=2, space="PSUM") as pctxT,  # 2 banks
            tc.tile_pool(name="ps2", bufs=1, space="PSUM") as ps2,      # 4 banks
        ):
            ident = persist.tile([128, 128], F32)
            make_identity(nc, ident[:])

            qn = persist.tile([128, L], F32)   # Q natural: part=lq%128, free=(t,d)
            kn = persist.tile([128, L], F32)
            v_sb = persist.tile([128, L], F32)  # V natural: part=kv%128, free=(t,dv)
            qt = persist.tile([128, L], F32)   # Q^T: part=d, free=lq
            kt = persist.tile([128, L], F32)   # K^T: part=d, free=kv
            ctxT = persist.tile([128, L], F32)  # ctx^T: part=dv, free=lq
            recs = persist.tile([128, NT], F32)  # 1/rowsum, col t = lq tile t

            for src, dst in ((q_d, qn), (k_d, kn)):
                nc.sync.dma_start(
                    dst[:].rearrange("p (t d) -> p t d", d=D),
                    src.rearrange("(t p) d -> p t d", p=128),
                )
            # V feeds the fp32r AV matmul directly: declare the transfer as
            # float32r on both sides (same bits; PE rounds on ingest).
            nc.sync.dma_start(
                r(v_sb[:]).rearrange("p (t d) -> p t d", d=D),
                v_d.rearrange("(t p) d -> p t d", p=128).bitcast(F32R),
            )

            # phase 0: build Q^T and K^T via PE transposes
            for src, dst in ((qn, qt), (kn, kt)):
                for t in range(NT):
                    tp = ptp.tile([128, 512], F32, tag="tp")
                    nc.tensor.transpose(tp[:, 0:128], src[:, ts(t, 128)], ident[:])
                    nc.vector.tensor_copy(r(dst[:, ts(t, 128)]), tp[:, 0:128])

            # phases 1 + 2 interleaved per 512-wide Lq chunk so the scheduler
            # can keep PE/ACT/DVE/DMA all busy:
            #   phase 1: S^T / exp / ctx^T accumulation (chunk c)
            #   phase 2: natural-layout S, fused exp+rowsum, normalize, W out
            #            (the 4 Lq tiles covered by chunk c)
            def phase2_tile(t):
                w_sb = w_pool.tile([128, L], F32, tag="w")
                sp = ps2.tile([128, L], F32, tag="s2")
                for j in range(4):
                    nc.tensor.matmul(sp[:, ds(j * 512, 512)],
                                     r(qt[:, ts(t, 128)]),
                                     r(kt[:, ds(j * 512, 512)]),
                                     start=True, stop=True)
                den = den_pool.tile([128, 1], F32, tag="den")
                nc.scalar.activation(w_sb[:], sp[:],
                                     mybir.ActivationFunctionType.Exp,
                                     scale=SCALE, accum_out=den[:])
                nc.vector.reciprocal(recs[:, ds(t, 1)], den[:])
                nc.vector.tensor_scalar_mul(w_sb[:], w_sb[:],
                                            recs[:, ds(t, 1)])
                nc.sync.dma_start(w_d[ts(t, 128), :], w_sb[:])

            for c in range(L // 512):
                lq = ds(c * 512, 512)
                ctxT_ps = pctxT.tile([128, 512], F32, tag="ctxT")
                for k in range(NT):
                    st = ptp.tile([128, 512], F32, tag="tp")
                    nc.tensor.matmul(st[:], r(kt[:, ts(k, 128)]), r(qt[:, lq]),
                                     start=True, stop=True)
                    ex = expst_pool.tile([128, 512], F32, tag="expst")
                    nc.scalar.activation(r(ex[:]), st[:],
                                         mybir.ActivationFunctionType.Exp,
                                         scale=SCALE)
                    nc.tensor.matmul(ctxT_ps[:], r(v_sb[:, ts(k, 128)]), r(ex[:]),
                                     start=(k == 0), stop=(k == NT - 1))
                    if k % 4 == 3:
                        phase2_tile(4 * c + k // 4)
                nc.vector.tensor_copy(ctxT[:, lq], ctxT_ps[:])

            # phase 3: ctx^T -> natural, normalize, out
            for t in range(NT):
                cp = ptp.tile([128, 512], F32, tag="tp")
                nc.tensor.transpose(cp[:, 0:128], ctxT[:, ts(t, 128)], ident[:])
                cs = cs_pool.tile([128, 128], F32, tag="cs")
                nc.vector.tensor_scalar_mul(cs[:], cp[:, 0:128], recs[:, ds(t, 1)])
                nc.sync.dma_start(ctx_d[ts(t, 128), :], cs[:])

    nc.compile()
    return nc


_NC_CACHE = None


def _get_nc():
    global _NC_CACHE
    if _NC_CACHE is None:
        _NC_CACHE = build_attention_nc()
    return _NC_CACHE


def kernel(query, key, value):
    query = np.asarray(query, dtype=np.float32)
    key = np.asarray(key, dtype=np.float32)
    value = np.asarray(value, dtype=np.float32)
    assert query.shape == (B, L, D), query.shape

    nc = _get_nc()
    in_maps = [
        {
            "q": np.ascontiguousarray(query[b]),
            "k": np.ascontiguousarray(key[b]),
            "v": np.ascontiguousarray(value[b]),
        }
        for b in range(B)
    ]
    res = bass_utils.run_bass_kernel_spmd(
        nc, in_maps, core_ids=list(range(N_CORES))
    )
    weights = np.stack([res.results[b]["w"] for b in range(B)])
    ctx = np.stack([res.results[b]["ctx"] for b in range(B)])
    return weights, ctx



# revision 17
# speedup vs baseline: 1.2628x; 1.2628x over previous
"""Batch-parallel attention (B=8, Lq=Lkv=2048, D=DV=128) for 8 Trainium2 NeuronCores.

Sharding: batch dim across the 8 cores (data parallel, no cross-core comms).
Each core computes, for its (2048, 128) Q/K/V slice:
    S = Q @ K^T * (1/sqrt(D));  W = softmax(S);  ctx = W @ V
and writes both W (2048, 2048) and ctx (2048, 128) in natural layout.

Per-core plan (matmuls in float32r / tf32 operand precision with fp32 PSUM
accumulation — full PE rate at N=512, ~4e-4 rel err vs the fp32 reference.
The two QK^T orientations are recomputed rather than transposed: PE streams
512 cols per 216 ns in a matmul vs 128 cols per ~275 ns in transpose mode,
so recompute is ~5x cheaper than transposing the 16 MiB score matrix):
  phase 0: load Q,K,V; build Q^T, K^T ([D, L] layout) via 32 PE transposes.
  phase 1 (S^T layout, Lq chunks of 512): S^T = (K^T)^T_blk @ Q^T -> exp on
    ScalarE (PSUM->SBUF) -> ctx^T += V_blk^T @ expS^T accumulated over the 16
    kv blocks in PSUM.  Produces unnormalized ctx^T [DV, L] in SBUF.
  phase 2 (natural layout, 16 Lq tiles, interleaved into phase 1):
    S = (Q^T)^T_blk @ K^T recomputed, one FD=2048 exp with fused
    per-partition row-sum (accum_out) -> reciprocal -> tensor_scalar
    normalize -> DMA out 1 MiB weight tiles.
  ctx finish (inlined per chunk): PE-transpose ctx^T tiles back to natural,
    normalize by the same reciprocals, DMA out.

softmax skips the max-subtraction: scores are ~N(0,1) after scaling, so
exp() stays in [e-6, e+6] — no overflow risk in fp32, and exp(x)/sum(exp(x))
is algebraically identical to the max-subtracted form.
"""

import math

import numpy as np

import concourse.mybir as mybir
import concourse.tile as tile
from concourse import bacc
from concourse import bass_utils
from concourse.bass import ds, ts
from concourse.masks import make_identity

B, L, D = 8, 2048, 128
N_CORES = 8
NT = L // 128          # 16 128-row tiles
SCALE = 1.0 / math.sqrt(D)
F32 = mybir.dt.float32
F32R = mybir.dt.float32r


def r(ap):
    """Reinterpret an fp32 AP as float32r (full-rate PE matmul mode)."""
    return ap.bitcast(F32R)


def build_attention_nc():
    nc = bacc.Bacc("TRN2", target_bir_lowering=False, debug=False,
                   num_devices=N_CORES)

    q_d = nc.dram_tensor("q", (L, D), F32, kind="ExternalInput")
    k_d = nc.dram_tensor("k", (L, D), F32, kind="ExternalInput")
    v_d = nc.dram_tensor("v", (L, D), F32, kind="ExternalInput")
    w_d = nc.dram_tensor("w", (L, L), F32, kind="ExternalOutput")
    ctx_d = nc.dram_tensor("ctx", (L, D), F32, kind="ExternalOutput")

    with tile.TileContext(nc) as tc:
        with (
            # persistent SBUF (one buf each)
            tc.tile_pool(name="persist", bufs=1) as persist,
            # cycling SBUF pools
            tc.tile_pool(name="expst", bufs=6) as expst_pool,
            tc.tile_pool(name="wout", bufs=4) as w_pool,
            tc.tile_pool(name="csml", bufs=8) as cs_pool,
            tc.tile_pool(name="dens", bufs=4) as den_pool,
            # PSUM pools: 3 + 1 + 4 = 8 banks total
            tc.tile_pool(name="ptp", bufs=3, space="PSUM") as ptp,      # 3 banks
            tc.tile_pool(name="pctxT", bufs=1, space="PSUM") as pctxT,  # 1 bank
            tc.tile_pool(name="ps2", bufs=1, space="PSUM") as ps2,      # 4 banks
        ):
            ident = persist.tile([128, 128], F32)
            make_identity(nc, ident[:])

            qn = persist.tile([128, L], F32)   # Q natural: part=lq%128, free=(t,d)
            kn = persist.tile([128, L], F32)
            v_sb = persist.tile([128, L], F32)  # V natural: part=kv%128, free=(t,dv)
            qt = persist.tile([128, L], F32)   # Q^T: part=d, free=lq
            kt = persist.tile([128, L], F32)   # K^T: part=d, free=kv
            ctxT = persist.tile([128, L], F32)  # ctx^T: part=dv, free=lq
            recs = persist.tile([128, NT], F32)  # 1/rowsum, col t = lq tile t

            # quarter-split loads let the first transposes start early
            for src, dst in ((q_d, qn), (k_d, kn)):
                for q4 in range(4):
                    nc.sync.dma_start(
                        dst[:, ds(q4 * 512, 512)].rearrange(
                            "p (t d) -> p t d", d=D),
                        src.rearrange("(t p) d -> p t d", p=128)[
                            :, ds(q4 * 4, 4), :],
                    )
            # V feeds the fp32r AV matmul directly: declare the transfer as
            # float32r on both sides (same bits; PE rounds on ingest).
            nc.sync.dma_start(
                r(v_sb[:]).rearrange("p (t d) -> p t d", d=D),
                v_d.rearrange("(t p) d -> p t d", p=128).bitcast(F32R),
            )

            # phase 0: build Q^T and K^T via PE transposes
            for src, dst in ((qn, qt), (kn, kt)):
                for t in range(NT):
                    tp = ptp.tile([128, 512], F32, tag="tp")
                    nc.tensor.transpose(tp[:, 0:128], src[:, ts(t, 128)], ident[:])
                    nc.vector.tensor_copy(r(dst[:, ts(t, 128)]), tp[:, 0:128])

            # phases 1 + 2 interleaved per 512-wide Lq chunk so the scheduler
            # can keep PE/ACT/DVE/DMA all busy:
            #   phase 1: S^T / exp / ctx^T accumulation (chunk c)
            #   phase 2: natural-layout S, fused exp+rowsum, normalize, W out
            #            (the 4 Lq tiles covered by chunk c)
            def phase2_tile(t):
                w_sb = w_pool.tile([128, L], F32, tag="w")
                sp = ps2.tile([128, L], F32, tag="s2")
                for j in range(4):
                    nc.tensor.matmul(sp[:, ds(j * 512, 512)],
                                     r(qt[:, ts(t, 128)]),
                                     r(kt[:, ds(j * 512, 512)]),
                                     start=True, stop=True)
                den = den_pool.tile([128, 1], F32, tag="den")
                nc.scalar.activation(w_sb[:], sp[:],
                                     mybir.ActivationFunctionType.Exp,
                                     scale=SCALE, accum_out=den[:])
                nc.vector.reciprocal(recs[:, ds(t, 1)], den[:])
                nc.vector.tensor_scalar_mul(w_sb[:], w_sb[:],
                                            recs[:, ds(t, 1)])
                nc.sync.dma_start(w_d[ts(t, 128), :], w_sb[:])

            for c in range(L // 512):
                lq = ds(c * 512, 512)
                ctxT_ps = pctxT.tile([128, 512], F32, tag="ctxT")
                for k in range(NT):
                    st = ptp.tile([128, 512], F32, tag="tp")
                    nc.tensor.matmul(st[:], r(kt[:, ts(k, 128)]), r(qt[:, lq]),
                                     start=True, stop=True)
                    ex = expst_pool.tile([128, 512], F32, tag="expst")
                    nc.scalar.activation(r(ex[:]), st[:],
                                         mybir.ActivationFunctionType.Exp,
                                         scale=SCALE)
                    nc.tensor.matmul(ctxT_ps[:], r(v_sb[:, ts(k, 128)]), r(ex[:]),
                                     start=(k == 0), stop=(k == NT - 1))
                    if k % 4 == 3:
                        phase2_tile(4 * c + k // 4)
                nc.vector.tensor_copy(ctxT[:, lq], ctxT_ps[:])
                # ctx finish for this chunk's 4 lq tiles (pipelines into the
                # next chunk's compute): transpose to natural, normalize, out
                for t3 in range(4 * c, 4 * c + 4):
                    cp = ptp.tile([128, 512], F32, tag="tp")
                    nc.tensor.transpose(cp[:, 0:128], ctxT[:, ts(t3, 128)],
                                        ident[:])
                    cs = cs_pool.tile([128, 128], F32, tag="cs")
                    nc.vector.tensor_scalar_mul(cs[:], cp[:, 0:128],
                                                recs[:, ds(t3, 1)])
                    nc.sync.dma_start(ctx_d[ts(t3, 128), :], cs[:])


    nc.compile()
    return nc


_NC_CACHE = None


def _get_nc():
    global _NC_CACHE
    if _NC_CACHE is None:
        _NC_CACHE = build_attention_nc()
    return _NC_CACHE


def kernel(query, key, value):
    query = np.asarray(query, dtype=np.float32)
    key = np.asarray(key, dtype=np.float32)
    value = np.asarray(value, dtype=np.float32)
    assert query.shape == (B, L, D), query.shape

    nc = _get_nc()
    in_maps = [
        {
            "q": np.ascontiguousarray(query[b]),
            "k": np.ascontiguousarray(key[b]),
            "v": np.ascontiguousarray(value[b]),
        }
        for b in range(B)
    ]
    res = bass_utils.run_bass_kernel_spmd(
        nc, in_maps, core_ids=list(range(N_CORES))
    )
    weights = np.stack([res.results[b]["w"] for b in range(B)])
    ctx = np.stack([res.results[b]["ctx"] for b in range(B)])
    return weights, ctx

